# revision 7
# baseline (speedup 1.0000x reference)
"""KoLeo loss kernel for Trainium2, 8 NeuronCores (SPMD, no collectives).

Math (reference):
  x = s / (||s||_2 + 1e-8)  row-normalize
  dots = x @ x.T,  diag masked; idx = argmax(dots, axis=1)
  d_i = ||x_i - x_idx[i]|| ; loss = -mean(log(d_i + 2e-8))

Strategy per core c (owns rows [c*1024, (c+1)*1024)):
  - inputs: full  s [8192,1024] f32 (shared), own block s_own [1024,1024] f32
  - build xT (normalized, transposed) [128p x 8dc x 8192] bf16 in SBUF:
    bf16 cast-DMA load of s row-chunks, ACT square+accum -> sumsq,
    PE "transpose" = chunk.T @ diag(1/(norm+eps))  (normalize fused into
    the transpose's streaming operand), ACT evacuates PSUM -> xT.
  - own rows likewise -> xT_own [128 x 8dc x 1024] bf16 (static offsets,
    so the compiled program is identical on all 8 cores).
  - dots row-tile [128 x 8192] = xT_own_i.T @ xT  (bf16, fp32 PSUM,
    8 K-chunks accumulated; 16 j-tiles of 512), ACT copies PSUM->SBUF bf16.
  - nc.vector.max/max_index top-8 over the 8192-wide row: rank-0 is the
    self dot (=1, strictly the max), rank-1 is the nearest neighbor.
  - indirect-DMA gather of NN raw rows from HBM, renormalize in fp32,
    exact fp32 distance vs renormalized own rows, ACT Ln(d + 2e-8).
  - output [128 x 8] per core; host: loss = -mean(all 8192 values).
"""

import os
import sys

import numpy as np

for _p in ("/opt/trn_rl_repo", "/root/.axon_site/_ro/trn_rl_repo"):
    if os.path.isdir(_p) and _p not in sys.path:
        sys.path.insert(0, _p)

N, D, M = 8192, 1024, 8
NO = N // M            # 1024 own rows per core
P = 128
RT = NO // P           # 8 own row-tiles
RC = N // P            # 64 row chunks of the full matrix
DC = D // P            # 8 contraction chunks
JW = 512               # j tile width (one PSUM bank)
JT = N // JW           # 16 j tiles
EPS = 1e-8

_CACHE = {}


def _hoist_waits(nc, mybir):
    """This walrus build rejects sync waits attached to compute/DMA/Drain
    instructions ("Too many sync wait commands"); hoist every attached wait
    into a standalone single-wait EventSemaphore right before the
    instruction, on the same engine."""
    for fn in nc.m.functions:
        for blk in fn.blocks:
            out = []
            for inst in blk.instructions:
                si = inst.sync_info
                if si is None or not len(si.on_wait):
                    out.append(inst)
                    continue
                if type(inst).__name__ == "InstEventSemaphore" and len(si.on_wait) == 1:
                    out.append(inst)
                    continue
                for k, w in enumerate(si.on_wait):
                    ev = mybir.InstEventSemaphore(name=f"{inst.name}.w{k}", ins=[], outs=[])
                    ev.engine = inst.engine
                    ev.sync_info = mybir.SyncInfo(on_wait=[w], on_update=[])
                    out.append(ev)
                inst.sync_info = mybir.SyncInfo(on_wait=[], on_update=list(si.on_update))
                out.append(inst)
            blk.instructions = out


def _build():
    import concourse.bass as bass
    import concourse.mybir as mybir
    import concourse.tile as tile
    from concourse.masks import make_identity

    fp32 = mybir.dt.float32
    bf16 = mybir.dt.bfloat16
    u32 = mybir.dt.uint32
    AF = mybir.ActivationFunctionType

    nc = bass.Bass()
    s_hbm = nc.dram_tensor("s", [N, D], fp32, kind="ExternalInput")
    so_hbm = nc.dram_tensor("s_own", [NO, D], fp32, kind="ExternalInput")
    out_hbm = nc.dram_tensor("out", [P, RT], fp32, kind="ExternalOutput")

    with tile.TileContext(nc) as tc:
        with (
            tc.tile_pool(name="big", bufs=1) as big,
            tc.tile_pool(name="sm", bufs=1) as sm,
            tc.tile_pool(name="ld", bufs=3) as ld,
            tc.tile_pool(name="scr", bufs=2) as scr,
            tc.tile_pool(name="gf", bufs=2) as gf,
            tc.tile_pool(name="smi", bufs=2) as smi,
            tc.tile_pool(name="psA", bufs=2, space="PSUM") as psA,
            tc.tile_pool(name="psB", bufs=6, space="PSUM") as psB,
        ):
            ident = sm.tile([P, P], bf16)
            make_identity(nc, ident[:])
            epsc = sm.tile([P, 2], fp32)
            nc.gpsimd.memset(epsc[:, 0:1], EPS)
            nc.gpsimd.memset(epsc[:, 1:2], 2 * EPS)

            xT = big.tile([P, DC, N], bf16)        # 128 KB/partition
            xTo = big.tile([P, DC, NO], bf16)      # 16 KB/partition
            loss_cols = sm.tile([P, RT], fp32)

            ss = sm.tile([P, RC], fp32)            # sumsq of full rows (bf16 data)
            nrm = sm.tile([P, RC], fp32)
            inv_f = sm.tile([P, RC], fp32)
            sso = sm.tile([P, RT], fp32)           # same for own block
            nrmo = sm.tile([P, RT], fp32)
            invo_f = sm.tile([P, RT], fp32)

            def norm_chunks(src, n_chunks, ss_t, nrm_t, invf_t, xT_t, grp):
                """bf16-load `n_chunks` 128-row chunks of `src`, sumsq, and
                PE-transpose with fused 1/(norm+eps) column scaling into xT_t."""
                for r in range(n_chunks):
                    sf = ld.tile([P, D], fp32, tag="sf32", name=f"sf{r}")
                    nc.sync.dma_start(
                        out=sf[:], in_=src[r * P : (r + 1) * P, :]
                    )
                    sb = scr.tile([P, D], bf16, tag="sbf", name=f"sbf{r}")
                    nc.gpsimd.tensor_copy(sb[:], sf[:])
                    nc.scalar.activation(
                        sf[:], sf[:], AF.Square,
                        accum_out=ss_t[:, r : r + 1],
                    )
                    nc.scalar.sqrt(nrm_t[:, r : r + 1], ss_t[:, r : r + 1])
                    nc.scalar.activation(
                        nrm_t[:, r : r + 1], nrm_t[:, r : r + 1], AF.Identity,
                        bias=epsc[:, 0:1],
                    )
                    nc.vector.reciprocal(invf_t[:, r : r + 1], nrm_t[:, r : r + 1])
                    diag = smi.tile([P, P], bf16, tag="diag", name=f"diag{r}")
                    nc.vector.tensor_scalar_mul(
                        diag[:], ident[:], invf_t[:, r : r + 1]
                    )
                    for half in range(2):
                        pt = psA.tile([P, 4 * P], fp32, tag="ptr", name=f"pt{r}_{half}")
                        for b in range(4):
                            blk = half * 4 + b
                            nc.tensor.matmul(
                                pt[:, b * P : (b + 1) * P],
                                lhsT=sb[:, blk * P : (blk + 1) * P],
                                rhs=diag[:],
                                start=True,
                                stop=True,
                            )
                        nc.scalar.copy(
                            xT_t[:, half * 4 : half * 4 + 4, r * P : (r + 1) * P],
                            pt[:].rearrange("p (a b) -> p a b", a=4),
                        )

            norm_chunks(so_hbm, RT, sso, nrmo, invo_f, xTo, 8)
            norm_chunks(s_hbm, RC, ss, nrm, inv_f, xT, 8)

            # ---- main dots + argmax + gather + distance, per own row-tile ----
            JGRP = 6
            for i in range(RT):
                dots = big.tile([P, N], bf16, tag="dots")
                for j0 in range(0, JT, JGRP):
                    j1 = min(j0 + JGRP, JT)
                    pts = [
                        psB.tile([P, JW], fp32, tag="pmm", name=f"pmm_{i}_{j}")
                        for j in range(j0, j1)
                    ]
                    for dc in range(DC):
                        for jj, j in enumerate(range(j0, j1)):
                            nc.tensor.matmul(
                                pts[jj][:],
                                lhsT=xTo[:, dc, i * P : (i + 1) * P],
                                rhs=xT[:, dc, j * JW : (j + 1) * JW],
                                start=(dc == 0),
                                stop=(dc == DC - 1),
                            )
                    for jj, j in enumerate(range(j0, j1)):
                        nc.scalar.copy(dots[:, j * JW : (j + 1) * JW], pts[jj][:])

                top8 = smi.tile([P, 8], bf16, tag="top8")
                idx8 = smi.tile([P, 8], u32, tag="idx8")
                nc.vector.max(top8[:], dots[:])
                nc.vector.max_index(idx8[:], top8[:], dots[:])

                # gather NN raw rows (idx rank-1; rank-0 is the self match)
                g = gf.tile([P, D], fp32, tag="g")
                nc.gpsimd.indirect_dma_start(
                    out=g[:],
                    out_offset=None,
                    in_=s_hbm[:, :],
                    in_offset=bass.IndirectOffsetOnAxis(ap=idx8[:, 1:2], axis=0),
                )
                so = gf.tile([P, D], fp32, tag="so")
                nc.sync.dma_start(out=so[:], in_=so_hbm[i * P : (i + 1) * P, :])

                sq2 = scr.tile([P, D], bf16, tag="sq2")
                vg = smi.tile([P, 4], fp32, tag="vg")  # cols: ssg, ssn, d2, d
                vn = smi.tile([P, 4], fp32, tag="vn")
                nc.scalar.activation(sq2[:], g[:], AF.Square, accum_out=vg[:, 0:1])
                nc.scalar.activation(sq2[:], so[:], AF.Square, accum_out=vn[:, 0:1])
                nc.scalar.sqrt(vg[:, 1:2], vg[:, 0:1])
                nc.scalar.sqrt(vn[:, 1:2], vn[:, 0:1])
                nc.scalar.activation(vg[:, 1:2], vg[:, 1:2], AF.Identity, bias=epsc[:, 0:1])
                nc.scalar.activation(vn[:, 1:2], vn[:, 1:2], AF.Identity, bias=epsc[:, 0:1])
                nc.vector.reciprocal(vg[:, 2:3], vg[:, 1:2])
                nc.vector.reciprocal(vn[:, 2:3], vn[:, 1:2])
                nc.scalar.mul(g[:], g[:], vg[:, 2:3])    # normalized NN (fp32)
                nc.scalar.mul(so[:], so[:], vn[:, 2:3])  # normalized own (fp32)
                nc.vector.tensor_tensor(
                    out=so[:], in0=so[:], in1=g[:], op=mybir.AluOpType.subtract
                )
                nc.scalar.activation(sq2[:], so[:], AF.Square, accum_out=vn[:, 2:3])
                nc.scalar.sqrt(vn[:, 3:4], vn[:, 2:3])
                nc.scalar.activation(
                    loss_cols[:, i : i + 1], vn[:, 3:4], AF.Ln, bias=epsc[:, 1:2]
                )

            nc.sync.dma_start(out=out_hbm[:, :], in_=loss_cols[:])

    _hoist_waits(nc, mybir)
    return nc


def kernel(student_output: np.ndarray) -> np.ndarray:
    from concourse.bass_utils import run_bass_kernel_spmd

    s = np.ascontiguousarray(student_output, dtype=np.float32)
    assert s.shape == (N, D)

    if "nc" not in _CACHE:
        _CACHE["nc"] = _build()
    nc = _CACHE["nc"]

    in_maps = [
        {"s": s, "s_own": np.ascontiguousarray(s[c * NO : (c + 1) * NO])}
        for c in range(M)
    ]
    res = run_bass_kernel_spmd(nc, in_maps, core_ids=list(range(M)))
    _CACHE["last_results"] = res
    total = np.float64(0.0)
    for r in res.results:
        total += np.asarray(r["out"], dtype=np.float64).sum()
    return np.float32(-(total / N))
